# revision 41
# baseline (speedup 1.0000x reference)
"""Trainium2 Bass kernel for the EnhancedGNNDetector (3x GCN + GAT + pool + MLP).

Strategy (8 NeuronCores, SPMD single program):
  - Nodes sharded contiguously: core c owns dsts [c*6250, (c+1)*6250).
  - Self-loop edges are NOT in the edge stream: each layer's self-loop
    contribution is added at evict time from SBUF-resident stashes of the
    local table blocks (fused into the existing dinv-scale op).
  - Real edges partitioned by dst owner, sorted by dst, grouped into
    128-edge chunks per (128-dst block, src-half).  Chunk counts padded to
    the cross-core max so one static program serves all cores.
  - Per layer: node features are dinv-scaled, cast fp16 (fp8 for the GAT
    table), written to a local DRAM table, AllGathered to a full table;
    dma_gather pulls edge source rows; a one-hot S matrix (built per chunk
    on DVE via tensor_scalar is_equal, which hits the 4x perf mode) turns
    the segment-sum into PE matmuls accumulating in PSUM.
  - src index is int16 for dma_gather, so tables are gathered in two
    halves (rows < 32768 and >= 32768).
  - GAT: table rows are 512B: [hg fp8e4 (256) | al_s f32 (4) | pad].
    al_d per edge is gathered from a [NPC,128] fp16 table (256B rows).
    leaky_relu is computed on DVE (scalar_tensor_tensor max(x, 0.2x)) so
    the Activation engine only ever needs the exp/relu/identity table set.
    Softmax uses a global per-head shift c = leaky(max al_s + max al_d).
    den is computed by appending the per-head exp weights as extra message
    columns (cols 256:260 of the fp8 row).
"""

import numpy as np
import concourse.bacc as bacc
import concourse.bass as bass
import concourse.mybir as mybir
import concourse.tile as tile
from concourse.bass_utils import run_bass_kernel_spmd

F16 = np.float16
N = 50000
E = 800000
NCORES = 8
NPC = N // NCORES            # 6250 nodes per core
NB = (NPC + 127) // 128      # 49 dst blocks per core
LASTB = NPC - 128 * (NB - 1)  # 106 rows in last block
HALF = 32768                 # int16 gather split
D_IN = 128
HID = 256
H3D = 256                    # L3 table cols (dinv*h2; W3 applied at evict3)
HEADS = 4
FH = 64
GROW = 512                   # GAT table row: 256 fp8 hg + 16B f32 al_s + pad
GDM = 260                    # GAT message cols: 256 hg + 4 exp
OUT = 8
NEG = 0.2
GBLK = 2                     # blocks per gather group
PH0B = 7                     # phase-0 blocks per DMA batch

fp8 = mybir.dt.float8e4
fp16 = mybir.dt.float16
fp32 = mybir.dt.float32
i16 = mybir.dt.int16
ALU = mybir.AluOpType
ACT = mybir.ActivationFunctionType


# --------------------------------------------------------------------------
# host-side schedule + per-core streams
# --------------------------------------------------------------------------

def _preprocess(x, edge_index):
    src = edge_index[0]
    dst = edge_index[1]
    deg = (np.bincount(dst, minlength=N) + 1).astype(np.float32)  # +1 self loop
    dinv = (1.0 / np.sqrt(deg)).astype(np.float32)

    order = np.argsort(dst, kind="stable")
    s_src, s_dst = src[order], dst[order]

    core = s_dst // NPC
    blk = (s_dst % NPC) // 128
    half = (s_src >= HALF).astype(np.int64)

    # edge lists per (core, block, half)
    key = (core * NB + blk) * 2 + half
    korder = np.argsort(key, kind="stable")   # stable: keeps dst order inside
    k_src, k_dst, k_key = s_src[korder], s_dst[korder], key[korder]
    bounds = np.searchsorted(k_key, np.arange(NCORES * NB * 2 + 1))
    cnt = (bounds[1:] - bounds[:-1]).reshape(NCORES, NB, 2)
    ch = -(-cnt // 128)
    CH = ch.max(axis=0)                        # [NB, 2] cross-core chunk counts

    # canonical chunk layout: per GBLK-block group, lo spans then hi spans
    groups = [tuple(range(g, min(g + GBLK, NB))) for g in range(0, NB, GBLK)]
    chunk_block = []          # block id per canonical chunk
    layout = []               # per group: (lo_start, lo_n, hi_start, hi_n, blocks)
    pos = 0
    for g in groups:
        lo_start = pos
        for b in g:
            chunk_block += [b] * int(CH[b, 0])
            pos += int(CH[b, 0])
        hi_start = pos
        for b in g:
            chunk_block += [b] * int(CH[b, 1])
            pos += int(CH[b, 1])
        layout.append((lo_start, hi_start - lo_start, hi_start, pos - hi_start, g))
    NCH = pos

    # per-core streams
    def wrap(stream):
        return np.ascontiguousarray(np.tile(stream.reshape(-1, 16).T.copy(), (8, 1)))

    idxs_all, idxd_all, dstrel_all = [], [], []
    for c in range(NCORES):
        idx_stream = np.zeros(NCH * 128, np.int16)
        idxd_stream = np.zeros(NCH * 128, np.int16)
        rel_stream = np.full(NCH * 128, -1.0, np.float32)
        for (lo_s, lo_n, hi_s, hi_n, g) in layout:
            for h, start in ((0, lo_s), (1, hi_s)):
                p = start * 128
                for b in g:
                    k = (c * NB + b) * 2 + h
                    e0, e1 = bounds[k], bounds[k + 1]
                    n = e1 - e0
                    cap = int(CH[b, h]) * 128
                    es, ed = k_src[e0:e1], k_dst[e0:e1]
                    idx_stream[p:p + n] = (es - (HALF if h else 0)).astype(np.int16)
                    idxd_stream[p:p + n] = (ed - c * NPC).astype(np.int16)
                    rel_stream[p:p + n] = (ed - c * NPC - b * 128).astype(np.float32)
                    p += cap
        idxs_all.append(wrap(idx_stream))
        idxd_all.append(wrap(idxd_stream))
        dstrel_all.append(rel_stream)

    dinv_blocks = []
    for c in range(NCORES):
        dv = np.ones(NB * 128, np.float32)
        dv[:NPC] = dinv[c * NPC:(c + 1) * NPC]
        dinv_blocks.append(np.ascontiguousarray(dv.reshape(NB, 128).T))  # [128, NB]

    return {
        "layout": layout, "chunk_block": chunk_block, "NCH": NCH, "CH": CH,
        "idxs": idxs_all, "idxd": idxd_all, "dstrel": dstrel_all, "dinv": dinv_blocks,
    }


# --------------------------------------------------------------------------
# device program
# --------------------------------------------------------------------------

def _build(sched, repeat=1, no_cc=False):
    NCH = sched["NCH"]
    layout = sched["layout"]
    chunk_block = sched["chunk_block"]

    nc = bacc.Bacc("TRN2", target_bir_lowering=False, debug=False,
                   num_devices=NCORES, num_swdge_queues=4)

    # ---------------- external tensors ----------------
    xs = nc.dram_tensor("xs", [NPC, D_IN], fp32, kind="ExternalInput")
    idxs_d = nc.dram_tensor("idxs_d", [128, NCH * 8], i16, kind="ExternalInput")
    dstrel_d = nc.dram_tensor("dstrel_d", [128, NCH], fp32, kind="ExternalInput")
    dinv_d = nc.dram_tensor("dinv_d", [128, NB], fp32, kind="ExternalInput")
    w1_d = nc.dram_tensor("w1_d", [128, HID], fp16, kind="ExternalInput")
    w2_d = nc.dram_tensor("w2_d", [128, 2 * HID], fp16, kind="ExternalInput")
    w3_d = nc.dram_tensor("w3_d", [128, 256], fp16, kind="ExternalInput")
    wg_d = nc.dram_tensor("wg_d", [128, HID], fp16, kind="ExternalInput")
    b1_d = nc.dram_tensor("b1_d", [1, HID], fp16, kind="ExternalInput")
    b2_d = nc.dram_tensor("b2_d", [1, HID], fp16, kind="ExternalInput")
    b3_d = nc.dram_tensor("b3_d", [128, 128], fp32, kind="ExternalInput")
    bg_d = nc.dram_tensor("bg_d", [128, HID], fp32, kind="ExternalInput")
    asrc_d = nc.dram_tensor("asrc_d", [128, HID], fp32, kind="ExternalInput")
    adst_d = nc.dram_tensor("adst_d", [128, HID], fp32, kind="ExternalInput")
    idxd_d = nc.dram_tensor("idxd_d", [128, NCH * 8], i16, kind="ExternalInput")
    wc1_d = nc.dram_tensor("wc1_d", [128, 2 * 128], fp32, kind="ExternalInput")
    wc2_d = nc.dram_tensor("wc2_d", [128, 64], fp32, kind="ExternalInput")
    wc3_d = nc.dram_tensor("wc3_d", [64, 8], fp32, kind="ExternalInput")
    bc1_d = nc.dram_tensor("bc1_d", [128, 1], fp32, kind="ExternalInput")
    bc2_d = nc.dram_tensor("bc2_d", [64, 1], fp32, kind="ExternalInput")
    bc3_d = nc.dram_tensor("bc3_d", [8, 1], fp32, kind="ExternalInput")
    rowmask_d = nc.dram_tensor("rowmask_d", [128, 1], fp32, kind="ExternalInput")
    out_d = nc.dram_tensor("out_d", [8, 1], fp32, kind="ExternalOutput")

    # internal DRAM tables
    g1loc = nc.dram_tensor("g1loc", [NPC, D_IN], fp16)
    g1full = nc.dram_tensor("g1full", [N, D_IN], fp16, addr_space="Shared")
    g2loc = nc.dram_tensor("g2loc", [NPC, HID], fp16)
    g2full = nc.dram_tensor("g2full", [N, HID], fp16, addr_space="Shared")
    g3loc = nc.dram_tensor("g3loc", [NPC, H3D], fp16)
    g3full = nc.dram_tensor("g3full", [N, H3D], fp16, addr_space="Shared")
    gtloc = nc.dram_tensor("gtloc", [NPC, GROW], fp8)
    gtfull = nc.dram_tensor("gtfull", [N, GROW], fp8, addr_space="Shared")
    aldtab = nc.dram_tensor("aldtab", [NPC, 128], fp16)
    arin = nc.dram_tensor("arin", [128, 2], fp32)
    arout = nc.dram_tensor("arout", [128, 2], fp32, addr_space="Shared")

    RG = [list(range(NCORES))]

    with tile.TileContext(nc) as tc:
        import contextlib
        es = contextlib.ExitStack()
        with es:
            pers = es.enter_context(tc.tile_pool(name="pers", bufs=1))
            # ---------- persistent SBUF ----------
            idxs = pers.tile([128, NCH * 8], i16)
            nc.sync.dma_start(idxs[:], idxs_d[:])
            dstrel = pers.tile([128, NCH], fp32)
            nc.sync.dma_start(dstrel[:], dstrel_d[:])
            dinv = pers.tile([128, NB], fp32)
            nc.sync.dma_start(dinv[:], dinv_d[:])

            w1 = pers.tile([128, HID], fp16); nc.sync.dma_start(w1[:], w1_d[:])
            w2 = pers.tile([128, 2 * HID], fp16); nc.sync.dma_start(w2[:], w2_d[:])
            w3 = pers.tile([128, 256], fp16); nc.sync.dma_start(w3[:], w3_d[:])
            wg = pers.tile([128, HID], fp16); nc.sync.dma_start(wg[:], wg_d[:])
            b1r = pers.tile([1, HID], fp16); nc.sync.dma_start(b1r[:], b1_d[:])
            b2r = pers.tile([1, HID], fp16); nc.sync.dma_start(b2r[:], b2_d[:])
            b3r = pers.tile([128, 128], fp32); nc.sync.dma_start(b3r[:], b3_d[:])
            bgr = pers.tile([128, HID], fp32); nc.sync.dma_start(bgr[:], bg_d[:])
            asr = pers.tile([128, HID], fp32)
            adr = pers.tile([128, HID], fp32)
            idxd = pers.tile([128, NCH * 8], i16)
            nc.sync.dma_start(idxd[:], idxd_d[:])
            wc1 = pers.tile([128, 2 * 128], fp32)
            wc2 = pers.tile([128, 64], fp32)
            wc3 = pers.tile([64, 8], fp32)
            bc1 = pers.tile([128, 1], fp32); nc.sync.dma_start(bc1[:], bc1_d[:])
            bc2 = pers.tile([64, 1], fp32); nc.sync.dma_start(bc2[:], bc2_d[:])
            bc3 = pers.tile([8, 1], fp32); nc.sync.dma_start(bc3[:], bc3_d[:])
            rowmask = pers.tile([128, 1], fp32); nc.sync.dma_start(rowmask[:], rowmask_d[:])

            iota_i = pers.tile([128, 128], i16)
            nc.gpsimd.iota(iota_i[:], pattern=[[1, 128]], base=0, channel_multiplier=0)
            iota_f = pers.tile([128, 128], fp16)
            nc.vector.tensor_copy(iota_f[:], iota_i[:])
            iop_i = pers.tile([128, 1], i16)
            nc.gpsimd.iota(iop_i[:], pattern=[[1, 1]], base=0, channel_multiplier=1)
            iop_f = pers.tile([128, 1], fp16)
            nc.vector.tensor_copy(iop_f[:], iop_i[:])
            ident = pers.tile([128, 128], fp16)
            nc.vector.tensor_tensor(
                ident[:], iop_f[:].broadcast_to([128, 128]), iota_f[:],
                op=ALU.is_equal)
            ones_r = pers.tile([1, 128], fp16)
            nc.vector.memset(ones_r[:], 1.0)
            ones_c = pers.tile([128, 1], fp16)
            nc.vector.memset(ones_c[:], 1.0)

            als_all = pers.tile([128, NB, HEADS], fp32)
            ald_all = pers.tile([128, NB, HEADS], fp32)
            crep = pers.tile([128, HEADS], fp32)

            # self-loop stashes: table row of each local node, pre-scaled by
            # dinv[d] (so evict just adds them to the PSUM aggregate).
            st1 = pers.tile([128, NB, D_IN], fp16)
            st2 = pers.tile([128, NB, HID], fp8)
            st3 = pers.tile([128, NB, H3D], fp8)
            sthg = pers.tile([128, NB, HID], fp8)

            h1_all = pers.tile([128, NB, HID], fp16)

            def rows(b):
                return LASTB if b == NB - 1 else 128

            # ---------- helpers ----------
            def transpose_to_sbuf(pool, psum_pool, src16, nslab, tag):
                """src16 [128, nslab*128] fp16 -> returns [128, nslab, 128] fp16."""
                out = pool.tile([128, nslab, 128], fp16, tag=tag, name=f"tT_{tag}")
                for s in range(nslab):
                    pt = psum_pool.tile([128, 128], fp16, tag="tr", name="pt_tr", bufs=2)
                    nc.tensor.transpose(pt[:], src16[:, s * 128:(s + 1) * 128], ident[:])
                    nc.scalar.copy(out[:, s, :], pt[:])
                return out

            qctr = [0]

            def next_q():
                qctr[0] += 1
                return qctr[0] % 4

            def gather_into(m_tile, table, start_chunk, n_chunks, elem):
                nc.gpsimd.dma_gather(
                    m_tile[:, 0:n_chunks, :], table,
                    idxs[:, start_chunk * 8:(start_chunk + n_chunks) * 8],
                    num_idxs=n_chunks * 128, num_idxs_reg=n_chunks * 128,
                    elem_size=elem, single_packet=False, queue_num=next_q())

            def build_S(pool, k0, n_chunks, tag, dtype=fp16, eng=None):
                S = pool.tile([128, n_chunks, 128], dtype, tag=tag, name=f"S_{tag}")
                for kk in range(n_chunks):
                    nc.vector.tensor_scalar(
                        S[:, kk, :], iota_f[:], dstrel[:, k0 + kk:k0 + kk + 1],
                        None, op0=ALU.is_equal)
                return S

            def maybe_cc(kind, op, replica_groups, ins, outs):
                if no_cc:
                    nc.sync.dma_start(outs[0].tensor[0:ins[0].shape[0]], ins[0])
                else:
                    nc.gpsimd.collective_compute(kind, op, replica_groups=replica_groups,
                                                 ins=ins, outs=outs)

            def run_body(rep):
                # ===== phase 0: g1 = dinv * x, st1 = dinv * g1, batched =====
                with tc.tile_pool(name=f"p0_{rep}", bufs=2) as p0:
                    for b0 in range(0, NB, PH0B):
                        nb_b = min(PH0B, NB - b0)
                        full = nb_b if b0 + nb_b < NB else nb_b - 1
                        xt = p0.tile([128, PH0B, D_IN], fp32, name="xt")
                        gt = p0.tile([128, PH0B, D_IN], fp16, name="gt")
                        if b0 + nb_b == NB:
                            nc.vector.memset(xt[:, nb_b - 1, :], 0.0)
                        r_tot = (nb_b - 1) * 128 + rows(b0 + nb_b - 1)
                        src_ap = xs[b0 * 128:b0 * 128 + r_tot, :]
                        if r_tot == nb_b * 128:
                            nc.sync.dma_start(
                                xt[:, 0:nb_b, :],
                                src_ap.rearrange("(b p) d -> p b d", p=128))
                        else:
                            if nb_b > 1:
                                nc.sync.dma_start(
                                    xt[:, 0:nb_b - 1, :],
                                    xs[b0 * 128:(b0 + nb_b - 1) * 128, :]
                                    .rearrange("(b p) d -> p b d", p=128))
                            nc.sync.dma_start(
                                xt[:LASTB, nb_b - 1, :],
                                xs[(NB - 1) * 128:NPC, :])
                        for j in range(nb_b):
                            b = b0 + j
                            nc.vector.tensor_scalar(
                                gt[:, j, :], xt[:, j, :], dinv[:, b:b + 1],
                                None, op0=ALU.mult)
                            nc.vector.tensor_scalar(
                                st1[:, b, :], gt[:, j, :], dinv[:, b:b + 1],
                                None, op0=ALU.mult)
                        if full > 0:
                            nc.sync.dma_start(
                                g1loc[b0 * 128:(b0 + full) * 128, :]
                                .rearrange("(b p) d -> p b d", p=128),
                                gt[:, 0:full, :])
                        if full < nb_b:
                            nc.sync.dma_start(
                                g1loc[(NB - 1) * 128:NPC, :], gt[:LASTB, nb_b - 1, :])

                if rep == 0:
                    nc.sync.dma_start(asr[:], asrc_d[:])
                    nc.sync.dma_start(adr[:], adst_d[:])
                    nc.sync.dma_start(wc1[:], wc1_d[:])
                    nc.sync.dma_start(wc2[:], wc2_d[:])
                    nc.sync.dma_start(wc3[:], wc3_d[:])
                maybe_cc("AllGather", ALU.bypass, RG, [g1loc[:]], [g1full[:]])

                # ================= GCN layer runner =================
                def gcn_layer(lname, table_full, D, evict_fn):
                    with (tc.tile_pool(name=f"{lname}_sb_{rep}", bufs=2) as lp,
                          tc.tile_pool(name=f"{lname}_ps_{rep}", bufs=5, space="PSUM") as pp,
                          tc.tile_pool(name=f"{lname}_wps_{rep}", bufs=2, space="PSUM") as wp):
                        tab_lo = table_full[0:HALF, :]
                        tab_hi = table_full[HALF:N, :]
                        for (lo_s, lo_n, hi_s, hi_n, g) in layout:
                            paggs = {}
                            for b in g:
                                paggs[b] = pp.tile([128, D], fp32, tag="agg", name="pagg")
                            first = {b: True for b in g}
                            total = {b: 0 for b in g}
                            for b in g:
                                total[b] = int(sched["CH"][b, 0] + sched["CH"][b, 1])
                            done = {b: 0 for b in g}
                            for (start, n_ch, tab) in ((lo_s, lo_n, tab_lo), (hi_s, hi_n, tab_hi)):
                                if n_ch == 0:
                                    continue
                                m = lp.tile([128, n_ch, D], fp16, tag="m", name="m", bufs=3)
                                gather_into(m, tab, start, n_ch, D)
                                S = build_S(lp, start, n_ch, "s")
                                for kk in range(n_ch):
                                    b = chunk_block[start + kk]
                                    done[b] += 1
                                    nc.tensor.matmul(
                                        paggs[b][:], S[:, kk, :], m[:, kk, :],
                                        start=first[b], stop=(done[b] == total[b]))
                                    first[b] = False
                            for b in g:
                                evict_fn(b, paggs[b], lp, wp)

                # ---------- layer 1 ----------
                def evict1(b, pagg, lp, wp):
                    r = rows(b)
                    a1s = lp.tile([128, D_IN], fp16, tag="ev1", name="a1s")
                    nc.vector.scalar_tensor_tensor(
                        a1s[:], pagg[:], dinv[:, b:b + 1], st1[:, b, :],
                        op0=ALU.mult, op1=ALU.add)
                    a1T = transpose_to_sbuf(lp, wp, a1s, 1, "ev1T")
                    ph = wp.tile([128, HID], fp32, tag="wout", name="ph1", bufs=1)
                    nc.tensor.matmul(ph[:], a1T[:, 0, :], w1[:], start=True, stop=False)
                    nc.tensor.matmul(ph[:], ones_r[:], b1r[:], start=False, stop=True)
                    h1t = h1_all[:, b, :]
                    nc.scalar.activation(h1t, ph[:], ACT.Relu)
                    g2t = lp.tile([128, HID], fp16, tag="ev1g", name="g2t")
                    nc.vector.tensor_scalar(g2t[:], h1t, dinv[:, b:b + 1], None, op0=ALU.mult)
                    nc.vector.tensor_scalar(st2[:, b, :], g2t[:], dinv[:, b:b + 1],
                                            None, op0=ALU.mult)
                    nc.sync.dma_start(g2loc[b * 128:b * 128 + r, :], g2t[:r, :])

                gcn_layer("L1", g1full, D_IN, evict1)
                maybe_cc("AllGather", ALU.bypass, RG, [g2loc[:]], [g2full[:]])

                # ---------- layer 2 (+ residual + L3 transform) ----------
                def evict2(b, pagg, lp, wp):
                    r = rows(b)
                    a2s = lp.tile([128, HID], fp16, tag="ev2", name="a2s")
                    nc.vector.scalar_tensor_tensor(
                        a2s[:], pagg[:], dinv[:, b:b + 1], st2[:, b, :],
                        op0=ALU.mult, op1=ALU.add)
                    a2T = transpose_to_sbuf(lp, wp, a2s, 2, "ev2T")
                    ph = wp.tile([128, HID], fp32, tag="wout", name="ph2", bufs=1)
                    nc.tensor.matmul(ph[:], a2T[:, 0, :], w2[:, 0:HID], start=True, stop=False)
                    nc.tensor.matmul(ph[:], a2T[:, 1, :], w2[:, HID:2 * HID], start=False, stop=False)
                    nc.tensor.matmul(ph[:], ones_r[:], b2r[:], start=False, stop=True)
                    r2 = lp.tile([128, HID], fp32, tag="ev2r", name="r2")
                    nc.scalar.activation(r2[:], ph[:], ACT.Relu)
                    h2t = lp.tile([128, HID], fp32, tag="ev2h", name="h2t")
                    nc.vector.tensor_tensor(h2t[:], r2[:], h1_all[:, b, :], op=ALU.add)
                    g3t = lp.tile([128, H3D], fp16, tag="ev2g", name="g3t")
                    nc.vector.tensor_scalar(g3t[:], h2t[:], dinv[:, b:b + 1], None, op0=ALU.mult)
                    nc.vector.tensor_scalar(st3[:, b, :], g3t[:], dinv[:, b:b + 1],
                                            None, op0=ALU.mult)
                    nc.sync.dma_start(g3loc[b * 128:b * 128 + r, :], g3t[:r, :])

                gcn_layer("L2", g2full, HID, evict2)
                maybe_cc("AllGather", ALU.bypass, RG, [g3loc[:]], [g3full[:]])

                # ---------- layer 3 aggregation + GAT prep ----------
                def evict3(b, pagg, lp, wp):
                    r = rows(b)
                    a3s = lp.tile([128, H3D], fp16, tag="ev3b", name="a3s")
                    nc.vector.scalar_tensor_tensor(
                        a3s[:], pagg[:], dinv[:, b:b + 1], st3[:, b, :],
                        op0=ALU.mult, op1=ALU.add)
                    a3T = transpose_to_sbuf(lp, wp, a3s, 2, "ev3aT")
                    p3 = wp.tile([128, 128], fp32, tag="wout", name="p3", bufs=1)
                    nc.tensor.matmul(p3[:], a3T[:, 0, :], w3[:, 0:128], start=True, stop=False)
                    nc.tensor.matmul(p3[:], a3T[:, 1, :], w3[:, 128:256], start=False, stop=True)
                    a3b = lp.tile([128, 128], fp32, tag="ev3c", name="a3b")
                    nc.vector.tensor_tensor(a3b[:], p3[:], b3r[:], op=ALU.add)
                    h3t16 = lp.tile([128, 128], fp16, tag="ev3h", name="h3t16")
                    nc.scalar.activation(h3t16[:], a3b[:], ACT.Relu)
                    h3T = transpose_to_sbuf(lp, wp, h3t16, 1, "ev3T")
                    phg = wp.tile([128, HID], fp32, tag="wout", name="phg", bufs=1)
                    nc.tensor.matmul(phg[:], h3T[:, 0, :], wg[:], start=True, stop=True)
                    # al_s / al_d
                    ts1 = lp.tile([128, HID], fp32, tag="ev3t1", name="ts1")
                    nc.vector.tensor_tensor(ts1[:], phg[:], asr[:], op=ALU.mult)
                    nc.vector.tensor_reduce(
                        als_all[:, b, :], ts1[:].rearrange("p (h f) -> p h f", h=HEADS),
                        axis=mybir.AxisListType.X, op=ALU.add)
                    ts2 = lp.tile([128, HID], fp32, tag="ev3t2", name="ts2")
                    nc.vector.tensor_tensor(ts2[:], phg[:], adr[:], op=ALU.mult)
                    nc.vector.tensor_reduce(
                        ald_all[:, b, :], ts2[:].rearrange("p (h f) -> p h f", h=HEADS),
                        axis=mybir.AxisListType.X, op=ALU.add)
                    # table tile: [hg fp8 (256) | al_s f32 (4) | pad]
                    tabt = lp.tile([128, GROW], fp8, tag="ev3tab", name="tabt")
                    nc.vector.tensor_copy(tabt[:, 0:HID], phg[:])
                    nc.vector.tensor_copy(sthg[:, b, :], tabt[:, 0:HID])
                    nc.scalar.copy(tabt[:, HID:HID + 16].bitcast(fp32), als_all[:, b, :])
                    nc.sync.dma_start(gtloc[b * 128:b * 128 + r, :], tabt[:r, :])
                    ald16 = lp.tile([128, HEADS], fp16, tag="ev3a", name="ald16")
                    nc.vector.tensor_copy(ald16[:], ald_all[:, b, :])
                    nc.sync.dma_start(aldtab[b * 128:b * 128 + r, 0:HEADS], ald16[:r, :])

                gcn_layer("L3", g3full, H3D, evict3)

                maybe_cc("AllGather", ALU.bypass, RG, [gtloc[:]], [gtfull[:]])

                # shift constants c[h] = leaky(max al_s + max al_d)
                cps = contextlib.ExitStack()
                cp = cps.enter_context(tc.tile_pool(name=f"cp_{rep}", bufs=1))
                cpp = cps.enter_context(tc.tile_pool(name=f"cpp_{rep}", bufs=1, space="PSUM"))
                m1 = cp.tile([128, HEADS], fp32)
                nc.vector.tensor_reduce(
                    m1[:], als_all[:].rearrange("p b h -> p h b"),
                    axis=mybir.AxisListType.X, op=ALU.max)
                m2 = cp.tile([128, HEADS], fp32)
                nc.vector.tensor_reduce(
                    m2[:], ald_all[:].rearrange("p b h -> p h b"),
                    axis=mybir.AxisListType.X, op=ALU.max)
                m1_16 = cp.tile([128, HEADS], fp16)
                nc.vector.tensor_copy(m1_16[:], m1[:])
                m2_16 = cp.tile([128, HEADS], fp16)
                nc.vector.tensor_copy(m2_16[:], m2[:])
                pmt1 = cpp.tile([HEADS, 128], fp16, tag="pmt1", name="pmt1")
                nc.tensor.transpose(pmt1[:], m1_16[:], ident[:])
                pmt2 = cpp.tile([HEADS, 128], fp16, tag="pmt2", name="pmt2")
                nc.tensor.transpose(pmt2[:], m2_16[:], ident[:])
                mt = cp.tile([HEADS, 2 * 128], fp32)
                nc.scalar.copy(mt[:, 0:128], pmt1[:])
                nc.scalar.copy(mt[:, 128:256], pmt2[:])
                ms = cp.tile([HEADS, 2], fp32)
                nc.vector.tensor_reduce(
                    ms[:], mt[:].rearrange("p (a j) -> p a j", a=2),
                    axis=mybir.AxisListType.X, op=ALU.max)
                ub = cp.tile([HEADS, 1], fp32)
                nc.vector.tensor_tensor(ub[:], ms[:, 0:1], ms[:, 1:2], op=ALU.add)
                ub2 = cp.tile([HEADS, 1], fp32)
                nc.vector.tensor_scalar(ub2[:], ub[:], 0.2, None, op0=ALU.mult)
                cc = cp.tile([HEADS, 1], fp32)
                nc.vector.tensor_tensor(cc[:], ub[:], ub2[:], op=ALU.max)
                cc16 = cp.tile([HEADS, 1], fp16)
                nc.vector.tensor_copy(cc16[:], cc[:])
                pcr = cpp.tile([1, HEADS], fp16)
                nc.tensor.transpose(pcr[:], cc16[:HEADS, :], ident[0:HEADS, 0:HEADS])
                pcr_sb = cp.tile([1, HEADS], fp16)
                nc.scalar.copy(pcr_sb[:], pcr[:])
                pcrep = cpp.tile([128, HEADS], fp32)
                nc.tensor.matmul(pcrep[:], ones_r[:], pcr_sb[:], start=True, stop=True)
                nc.scalar.copy(crep[:], pcrep[:])
                cps.close()

                # ================= GAT layer =================
                plp_cm = tc.tile_pool(name=f"pool_ps_{rep}", bufs=1, space="PSUM")
                plp = plp_cm.__enter__()
                ppool0 = plp.tile([128, 1], fp32, tag="pp0", name="ppool0")
                ppool1 = plp.tile([128, 1], fp32, tag="pp1", name="ppool1")
                with (tc.tile_pool(name=f"gat_sb_{rep}", bufs=2) as gp,
                      tc.tile_pool(name=f"gat_ps_{rep}", bufs=6, space="PSUM") as gpp):
                    tab_lo = gtfull[0:HALF, :]
                    tab_hi = gtfull[HALF:N, :]
                    first = {b: True for b in range(NB)}
                    done = {b: 0 for b in range(NB)}
                    total = {b: int(sched["CH"][b, 0] + sched["CH"][b, 1]) for b in range(NB)}
                    paggs = {}

                    def gat_span(start, n_ch, tab):
                        if n_ch == 0:
                            return
                        mald = gp.tile([128, n_ch, 128], fp16, tag="gald", name="mald", bufs=2)
                        nc.gpsimd.dma_gather(
                            mald[:, 0:n_ch, :], aldtab[:],
                            idxd[:, start * 8:(start + n_ch) * 8],
                            num_idxs=n_ch * 128, num_idxs_reg=n_ch * 128,
                            elem_size=128, single_packet=False, queue_num=next_q())
                        m = gp.tile([128, n_ch, GROW], fp8, tag="gm", name="gm", bufs=3)
                        gather_into(m, tab, start, n_ch, GROW)
                        S = build_S(gp, start, n_ch, "gs", dtype=fp8)
                        u = gp.tile([128, n_ch * HEADS], fp32, tag="gu", name="gu")
                        nc.vector.tensor_tensor(
                            u[:].rearrange("p (k h) -> p k h", h=HEADS),
                            m[:, :, HID:HID + 16].bitcast(fp32),
                            mald[:, :, 0:HEADS], op=ALU.add)
                        lsh = gp.tile([128, n_ch * HEADS], fp32, tag="glsh", name="glsh")
                        # leaky_relu on DVE: max(u, 0.2u), then subtract shift
                        nc.vector.scalar_tensor_tensor(
                            lsh[:], u[:], NEG, u[:], op0=ALU.mult, op1=ALU.max)
                        nc.vector.tensor_tensor(
                            lsh[:].rearrange("p (k h) -> p k h", h=HEADS),
                            lsh[:].rearrange("p (k h) -> p k h", h=HEADS),
                            crep[:].unsqueeze(1).broadcast_to([128, n_ch, HEADS]),
                            op=ALU.subtract)
                        expe = gp.tile([128, n_ch, HEADS], fp16, tag="gex", name="gex")
                        nc.scalar.activation(
                            expe[:].rearrange("p k h -> p (k h)"), lsh[:], ACT.Exp)
                        # write exp weights as message cols + weight hg in place
                        nc.vector.tensor_copy(m[:, :, HID:HID + HEADS], expe[:])
                        nc.vector.tensor_tensor(
                            m[:, :, 0:HID].rearrange("p k (h f) -> p k h f", h=HEADS),
                            m[:, :, 0:HID].rearrange("p k (h f) -> p k h f", h=HEADS),
                            expe[:].unsqueeze(3).broadcast_to([128, n_ch, HEADS, FH]),
                            op=ALU.mult)
                        for kk in range(n_ch):
                            b = chunk_block[start + kk]
                            done[b] += 1
                            nc.tensor.matmul(
                                paggs[b][:], S[:, kk, :], m[:, kk, 0:GDM],
                                start=first[b], stop=(done[b] == total[b]))
                            first[b] = False

                    def gat_evict(b):
                        r = rows(b)
                        pg = paggs.pop(b)
                        # self-loop attention weight wexp = exp(leaky(als+ald)-c)
                        wu = gp.tile([128, HEADS], fp32, tag="gwu", name="gwu")
                        nc.vector.tensor_tensor(
                            wu[:], als_all[:, b, :], ald_all[:, b, :], op=ALU.add)
                        nc.vector.scalar_tensor_tensor(
                            wu[:], wu[:], NEG, wu[:], op0=ALU.mult, op1=ALU.max)
                        nc.vector.tensor_tensor(wu[:], wu[:], crep[:], op=ALU.subtract)
                        wexp = gp.tile([128, HEADS], fp16, tag="gwe", name="gwe")
                        nc.scalar.activation(wexp[:], wu[:], ACT.Exp)
                        den = gp.tile([128, HEADS], fp32, tag="gden", name="gden")
                        nc.vector.tensor_tensor(
                            den[:], pg[:, HID:HID + HEADS], wexp[:], op=ALU.add)
                        nc.vector.tensor_scalar(den[:], den[:], 1e-30, None, op0=ALU.max)
                        rden = gp.tile([128, HEADS], fp32, tag="grden", name="grden")
                        nc.vector.reciprocal(rden[:], den[:])
                        t1 = gp.tile([128, HID], fp32, tag="gt1", name="gt1")
                        for h in range(HEADS):
                            nc.vector.scalar_tensor_tensor(
                                t1[:, h * FH:(h + 1) * FH],
                                sthg[:, b, h * FH:(h + 1) * FH],
                                wexp[:, h:h + 1],
                                pg[:, h * FH:(h + 1) * FH],
                                op0=ALU.mult, op1=ALU.add)
                        t2 = gp.tile([128, HID], fp32, tag="gt2", name="gt2")
                        nc.vector.tensor_tensor(
                            t2[:].rearrange("p (h f) -> p h f", h=HEADS),
                            t1[:].rearrange("p (h f) -> p h f", h=HEADS),
                            rden[:].unsqueeze(2).broadcast_to([128, HEADS, FH]),
                            op=ALU.mult)
                        nc.vector.tensor_tensor(t2[:], t2[:], bgr[:], op=ALU.add)
                        hatt = gp.tile([128, HID], fp16, tag="ghat", name="ghat")
                        nc.scalar.activation(hatt[:], t2[:], ACT.Relu)
                        if r < 128:
                            nc.vector.tensor_scalar(hatt[:], hatt[:], rowmask[:], None, op0=ALU.mult)
                        nc.tensor.matmul(ppool0[:], hatt[:, 0:128], ones_c[:],
                                         start=(b == 0), stop=(b == NB - 1))
                        nc.tensor.matmul(ppool1[:], hatt[:, 128:256], ones_c[:],
                                         start=(b == 0), stop=(b == NB - 1))

                    for (lo_s, lo_n, hi_s, hi_n, g) in layout:
                        for b in g:
                            paggs[b] = gpp.tile([128, GDM], fp32, tag="gagg", name="gagg")
                        for col, (start, n_ch) in enumerate(((lo_s, lo_n), (hi_s, hi_n))):
                            gat_span(start, n_ch, tab_lo if col == 0 else tab_hi)
                        for b in g:
                            gat_evict(b)

                # ---------- pooling + AllReduce + MLP ----------
                with (tc.tile_pool(name=f"mlp_sb_{rep}", bufs=1) as mp,
                      tc.tile_pool(name=f"mlp_ps_{rep}", bufs=1, space="PSUM") as mpp):
                    pool_sb = mp.tile([128, 2], fp32, name="pool_sb")
                    nc.scalar.copy(pool_sb[:, 0:1], ppool0[:])
                    nc.scalar.copy(pool_sb[:, 1:2], ppool1[:])
                    nc.sync.dma_start(arin[:], pool_sb[:])
                    maybe_cc("AllReduce", ALU.add, RG, [arin[:]], [arout[:]])
                    pooled = mp.tile([128, 2], fp32, name="pooled")
                    nc.sync.dma_start(pooled[:], arout[:])
                    nc.vector.tensor_scalar(pooled[:], pooled[:], 1.0 / N, None, op0=ALU.mult)
                    pz1 = mpp.tile([128, 1], fp32, tag="pz", name="pz1")
                    nc.tensor.matmul(pz1[:], wc1[:, 0:128], pooled[:, 0:1], start=True, stop=False)
                    nc.tensor.matmul(pz1[:], wc1[:, 128:256], pooled[:, 1:2], start=False, stop=True)
                    z1 = mp.tile([128, 1], fp32, name="z1")
                    nc.scalar.activation(z1[:], pz1[:], ACT.Relu, bias=bc1[:])
                    pz2 = mpp.tile([64, 1], fp32, tag="pz", name="pz2")
                    nc.tensor.matmul(pz2[:], wc2[:], z1[:], start=True, stop=True)
                    z2 = mp.tile([64, 1], fp32, name="z2")
                    nc.scalar.activation(z2[:], pz2[:], ACT.Relu, bias=bc2[:])
                    pz3 = mpp.tile([8, 1], fp32, tag="pz", name="pz3")
                    nc.tensor.matmul(pz3[:], wc3[:], z2[:64, :], start=True, stop=True)
                    zo = mp.tile([8, 1], fp32, name="zo")
                    nc.scalar.activation(zo[:], pz3[:], ACT.Identity, bias=bc3[:])
                    nc.sync.dma_start(out_d[:], zo[:])
                plp_cm.__exit__(None, None, None)

            for _rep in range(repeat):
                run_body(_rep)

    nc.compile()
    return nc


# --------------------------------------------------------------------------
# entry point
# --------------------------------------------------------------------------

def kernel(**inputs):
    x = np.asarray(inputs["x"], dtype=np.float32)
    ei = np.asarray(inputs["edge_index"], dtype=np.int64)
    sched = _preprocess(x, ei)
    nc = _build(sched)

    W = {k: np.asarray(v, dtype=np.float32) for k, v in inputs.items()
         if k not in ("x", "edge_index")}

    def pack_k(w, nslab):   # [K, M] -> [128, nslab*M] (row-slab packed)
        K, M = w.shape
        out = np.zeros((128, nslab * M), np.float32)
        for s in range(nslab):
            r0 = s * 128
            r1 = min(K, r0 + 128)
            out[0:r1 - r0, s * M:(s + 1) * M] = w[r0:r1]
        return out

    asrc_flat = np.tile(W["a_src"].reshape(1, HEADS * FH), (128, 1))
    adst_flat = np.tile(W["a_dst"].reshape(1, HEADS * FH), (128, 1))

    common = {
        "w1_d": pack_k(W["W1"], 1).astype(F16),
        "w2_d": pack_k(W["W2"], 2).astype(F16),
        "w3_d": pack_k(W["W3"], 2).astype(F16),
        "wg_d": pack_k(W["Wg"], 1).astype(F16),
        "b1_d": W["b1"].reshape(1, -1).astype(F16),
        "b2_d": W["b2"].reshape(1, -1).astype(F16),
        "b3_d": np.tile(W["b3"].reshape(1, -1), (128, 1)).astype(np.float32),
        "bg_d": np.tile(W["bg"].reshape(1, -1), (128, 1)).astype(np.float32),
        "asrc_d": asrc_flat.astype(np.float32),
        "adst_d": adst_flat.astype(np.float32),
        "wc1_d": pack_k(W["Wc1"], 2).astype(np.float32),
        "wc2_d": pack_k(W["Wc2"], 1)[:, :64].astype(np.float32),
        "wc3_d": pack_k(W["Wc3"], 1)[:64, :8].astype(np.float32),
        "bc1_d": W["bc1"].reshape(-1, 1).astype(np.float32),
        "bc2_d": W["bc2"].reshape(-1, 1).astype(np.float32),
        "bc3_d": W["bc3"].reshape(-1, 1).astype(np.float32),
        "rowmask_d": (np.arange(128) < LASTB).astype(np.float32).reshape(128, 1),
    }

    NCH = sched["NCH"]
    in_maps = []
    for c in range(NCORES):
        rel = sched["dstrel"][c]
        in_maps.append(dict(
            common,
            xs=np.ascontiguousarray(x[c * NPC:(c + 1) * NPC]),
            idxs_d=sched["idxs"][c],
            idxd_d=sched["idxd"][c],
            dstrel_d=np.ascontiguousarray(rel.reshape(NCH, 128).T).astype(np.float32),
            dinv_d=sched["dinv"][c],
        ))

    res = run_bass_kernel_spmd(nc, in_maps, core_ids=list(range(NCORES)))
    global LAST_RESULT
    LAST_RESULT = res
    return res.results[0]["out_d"].reshape(1, OUT).astype(np.float32)


LAST_RESULT = None


# revision 44
# speedup vs baseline: 1.0291x; 1.0291x over previous
"""Trainium2 Bass kernel for the EnhancedGNNDetector (3x GCN + GAT + pool + MLP).

Strategy (8 NeuronCores, SPMD single program):
  - Nodes sharded contiguously: core c owns dsts [c*6250, (c+1)*6250).
  - Self-loop edges are NOT in the edge stream: each layer's self-loop
    contribution is added at evict time from SBUF-resident stashes of the
    local table blocks (fused into the existing dinv-scale op).
  - Real edges partitioned by dst owner, sorted by dst, grouped into
    128-edge chunks per (128-dst block, src-half).  Chunk counts padded to
    the cross-core max so one static program serves all cores.
  - Per layer: node features are dinv-scaled, cast fp16 (fp8 for the GAT
    table), written to a local DRAM table, AllGathered to a full table;
    dma_gather pulls edge source rows; a one-hot S matrix (built per chunk
    on DVE via tensor_scalar is_equal, which hits the 4x perf mode) turns
    the segment-sum into PE matmuls accumulating in PSUM.
  - src index is int16 for dma_gather, so tables are gathered in two
    halves (rows < 32768 and >= 32768).
  - GAT: table rows are 512B: [hg fp8e4 (256) | al_s f32 (4) | pad].
    al_d per edge is gathered from a [NPC,128] fp16 table (256B rows).
    leaky_relu is computed on DVE (scalar_tensor_tensor max(x, 0.2x)) so
    the Activation engine only ever needs the exp/relu/identity table set.
    Softmax uses a global per-head shift c = leaky(max al_s + max al_d).
    den is computed by appending the per-head exp weights as extra message
    columns (cols 256:260 of the fp8 row).
"""

import numpy as np
import concourse.bacc as bacc
import concourse.bass as bass
import concourse.mybir as mybir
import concourse.tile as tile
from concourse.bass_utils import run_bass_kernel_spmd

F16 = np.float16
N = 50000
E = 800000
NCORES = 8
NPC = N // NCORES            # 6250 nodes per core
NB = (NPC + 127) // 128      # 49 dst blocks per core
LASTB = NPC - 128 * (NB - 1)  # 106 rows in last block
HALF = 32768                 # int16 gather split
D_IN = 128
HID = 256
H3D = 256                    # L3 table cols (dinv*h2; W3 applied at evict3)
HEADS = 4
FH = 64
GROW = 512                   # GAT table row: 256 fp8 hg + 16B f32 al_s + pad
GDM = 260                    # GAT message cols: 256 hg + 4 exp
OUT = 8
NEG = 0.2
GBLK = 2                     # blocks per gather group (GCN passes)
GATBLK = 3                   # blocks per gather group (GAT pass)
PH0B = 7                     # phase-0 blocks per DMA batch

fp8 = mybir.dt.float8e4
fp16 = mybir.dt.float16
fp32 = mybir.dt.float32
i16 = mybir.dt.int16
ALU = mybir.AluOpType
ACT = mybir.ActivationFunctionType


# --------------------------------------------------------------------------
# host-side schedule + per-core streams
# --------------------------------------------------------------------------

def _preprocess(x, edge_index):
    src = edge_index[0]
    dst = edge_index[1]
    deg = (np.bincount(dst, minlength=N) + 1).astype(np.float32)  # +1 self loop
    dinv = (1.0 / np.sqrt(deg)).astype(np.float32)

    order = np.argsort(dst, kind="stable")
    s_src, s_dst = src[order], dst[order]

    core = s_dst // NPC
    blk = (s_dst % NPC) // 128
    half = (s_src >= HALF).astype(np.int64)

    # edge lists per (core, block, half)
    key = (core * NB + blk) * 2 + half
    korder = np.argsort(key, kind="stable")   # stable: keeps dst order inside
    k_src, k_dst, k_key = s_src[korder], s_dst[korder], key[korder]
    bounds = np.searchsorted(k_key, np.arange(NCORES * NB * 2 + 1))
    cnt = (bounds[1:] - bounds[:-1]).reshape(NCORES, NB, 2)
    ch = -(-cnt // 128)
    CH = ch.max(axis=0)                        # [NB, 2] cross-core chunk counts

    # canonical chunk layouts: per-group (lo spans then hi spans); two group
    # sizes — GBLK for the GCN passes, GATBLK for the GAT pass (the padding is
    # per (block, half), so NCH is identical for any grouping).
    def mk_layout(blk_sz):
        groups = [tuple(range(g, min(g + blk_sz, NB))) for g in range(0, NB, blk_sz)]
        chunk_block = []
        layout = []
        pos = 0
        for g in groups:
            lo_start = pos
            for b in g:
                chunk_block += [b] * int(CH[b, 0])
                pos += int(CH[b, 0])
            hi_start = pos
            for b in g:
                chunk_block += [b] * int(CH[b, 1])
                pos += int(CH[b, 1])
            layout.append((lo_start, hi_start - lo_start, hi_start, pos - hi_start, g))
        return layout, chunk_block, pos

    layout, chunk_block, NCH = mk_layout(GBLK)
    layout_g, chunk_block_g, NCH_g = mk_layout(GATBLK)
    assert NCH_g == NCH

    # per-core streams
    def wrap(stream):
        return np.ascontiguousarray(np.tile(stream.reshape(-1, 16).T.copy(), (8, 1)))

    def mk_streams(c, lay, want_idxd):
        idx_stream = np.zeros(NCH * 128, np.int16)
        idxd_stream = np.zeros(NCH * 128, np.int16)
        rel_stream = np.full(NCH * 128, -1.0, np.float32)
        for (lo_s, lo_n, hi_s, hi_n, g) in lay:
            for h, start in ((0, lo_s), (1, hi_s)):
                p = start * 128
                for b in g:
                    k = (c * NB + b) * 2 + h
                    e0, e1 = bounds[k], bounds[k + 1]
                    n = e1 - e0
                    cap = int(CH[b, h]) * 128
                    es, ed = k_src[e0:e1], k_dst[e0:e1]
                    idx_stream[p:p + n] = (es - (HALF if h else 0)).astype(np.int16)
                    if want_idxd:
                        idxd_stream[p:p + n] = (ed - c * NPC).astype(np.int16)
                    rel_stream[p:p + n] = (ed - c * NPC - b * 128).astype(np.float32)
                    p += cap
        return idx_stream, idxd_stream, rel_stream

    idxs_all, idxd_all, dstrel_all = [], [], []
    idxs_g_all, dstrel_g_all = [], []
    for c in range(NCORES):
        i1, _, r1 = mk_streams(c, layout, False)
        i3, d3, r3 = mk_streams(c, layout_g, True)
        idxs_all.append(wrap(i1))
        dstrel_all.append(r1)
        idxs_g_all.append(wrap(i3))
        idxd_all.append(wrap(d3))
        dstrel_g_all.append(r3)

    dinv_blocks = []
    for c in range(NCORES):
        dv = np.ones(NB * 128, np.float32)
        dv[:NPC] = dinv[c * NPC:(c + 1) * NPC]
        dinv_blocks.append(np.ascontiguousarray(dv.reshape(NB, 128).T))  # [128, NB]

    return {
        "layout": layout, "chunk_block": chunk_block, "NCH": NCH, "CH": CH,
        "layout_g": layout_g, "chunk_block_g": chunk_block_g,
        "idxs": idxs_all, "idxd": idxd_all, "dstrel": dstrel_all,
        "idxs_g": idxs_g_all, "dstrel_g": dstrel_g_all, "dinv": dinv_blocks,
    }


# --------------------------------------------------------------------------
# device program
# --------------------------------------------------------------------------

def _build(sched, repeat=1, no_cc=False):
    NCH = sched["NCH"]
    layout = sched["layout"]
    chunk_block = sched["chunk_block"]

    layout_g = sched["layout_g"]
    chunk_block_g = sched["chunk_block_g"]

    nc = bacc.Bacc("TRN2", target_bir_lowering=False, debug=False,
                   num_devices=NCORES, num_swdge_queues=4)

    # ---------------- external tensors ----------------
    xs = nc.dram_tensor("xs", [NPC, D_IN], fp32, kind="ExternalInput")
    idxs_d = nc.dram_tensor("idxs_d", [128, NCH * 8], i16, kind="ExternalInput")
    dstrel_d = nc.dram_tensor("dstrel_d", [128, NCH], fp32, kind="ExternalInput")
    dinv_d = nc.dram_tensor("dinv_d", [128, NB], fp32, kind="ExternalInput")
    w1_d = nc.dram_tensor("w1_d", [128, HID], fp16, kind="ExternalInput")
    w2_d = nc.dram_tensor("w2_d", [128, 2 * HID], fp16, kind="ExternalInput")
    w3_d = nc.dram_tensor("w3_d", [128, 256], fp16, kind="ExternalInput")
    wg_d = nc.dram_tensor("wg_d", [128, HID], fp16, kind="ExternalInput")
    b1_d = nc.dram_tensor("b1_d", [1, HID], fp16, kind="ExternalInput")
    b2_d = nc.dram_tensor("b2_d", [1, HID], fp16, kind="ExternalInput")
    b3_d = nc.dram_tensor("b3_d", [128, 128], fp16, kind="ExternalInput")
    bg_d = nc.dram_tensor("bg_d", [128, HID], fp16, kind="ExternalInput")
    asrc_d = nc.dram_tensor("asrc_d", [128, HID], fp16, kind="ExternalInput")
    adst_d = nc.dram_tensor("adst_d", [128, HID], fp16, kind="ExternalInput")
    idxd_d = nc.dram_tensor("idxd_d", [128, NCH * 8], i16, kind="ExternalInput")
    idxs_g_d = nc.dram_tensor("idxs_g_d", [128, NCH * 8], i16, kind="ExternalInput")
    dstrel_g_d = nc.dram_tensor("dstrel_g_d", [128, NCH], fp32, kind="ExternalInput")
    wc1_d = nc.dram_tensor("wc1_d", [128, 2 * 128], fp16, kind="ExternalInput")
    wc2_d = nc.dram_tensor("wc2_d", [128, 64], fp32, kind="ExternalInput")
    wc3_d = nc.dram_tensor("wc3_d", [64, 8], fp32, kind="ExternalInput")
    bc1_d = nc.dram_tensor("bc1_d", [128, 1], fp32, kind="ExternalInput")
    bc2_d = nc.dram_tensor("bc2_d", [64, 1], fp32, kind="ExternalInput")
    bc3_d = nc.dram_tensor("bc3_d", [8, 1], fp32, kind="ExternalInput")
    rowmask_d = nc.dram_tensor("rowmask_d", [128, 1], fp32, kind="ExternalInput")
    out_d = nc.dram_tensor("out_d", [8, 1], fp32, kind="ExternalOutput")

    # internal DRAM tables
    g1loc = nc.dram_tensor("g1loc", [NPC, D_IN], fp16)
    g1full = nc.dram_tensor("g1full", [N, D_IN], fp16, addr_space="Shared")
    g2loc = nc.dram_tensor("g2loc", [NPC, HID], fp16)
    g2full = nc.dram_tensor("g2full", [N, HID], fp16, addr_space="Shared")
    g3loc = nc.dram_tensor("g3loc", [NPC, H3D], fp16)
    g3full = nc.dram_tensor("g3full", [N, H3D], fp16, addr_space="Shared")
    gtloc = nc.dram_tensor("gtloc", [NPC, GROW], fp8)
    gtfull = nc.dram_tensor("gtfull", [N, GROW], fp8, addr_space="Shared")
    aldtab = nc.dram_tensor("aldtab", [NPC, 128], fp16)
    arin = nc.dram_tensor("arin", [128, 2], fp32)
    arout = nc.dram_tensor("arout", [128, 2], fp32, addr_space="Shared")

    RG = [list(range(NCORES))]

    with tile.TileContext(nc) as tc:
        import contextlib
        es = contextlib.ExitStack()
        with es:
            pers = es.enter_context(tc.tile_pool(name="pers", bufs=1))
            # ---------- persistent SBUF ----------
            idxs = pers.tile([128, NCH * 8], i16)
            nc.sync.dma_start(idxs[:], idxs_d[:])
            dstrel = pers.tile([128, NCH], fp32)
            nc.sync.dma_start(dstrel[:], dstrel_d[:])
            dinv = pers.tile([128, NB], fp32)
            nc.sync.dma_start(dinv[:], dinv_d[:])

            w1 = pers.tile([128, HID], fp16); nc.sync.dma_start(w1[:], w1_d[:])
            w2 = pers.tile([128, 2 * HID], fp16); nc.sync.dma_start(w2[:], w2_d[:])
            w3 = pers.tile([128, 256], fp16); nc.sync.dma_start(w3[:], w3_d[:])
            wg = pers.tile([128, HID], fp16); nc.sync.dma_start(wg[:], wg_d[:])
            b1r = pers.tile([1, HID], fp16); nc.sync.dma_start(b1r[:], b1_d[:])
            b2r = pers.tile([1, HID], fp16); nc.sync.dma_start(b2r[:], b2_d[:])
            b3r = pers.tile([128, 128], fp16); nc.sync.dma_start(b3r[:], b3_d[:])
            bgr = pers.tile([128, HID], fp16); nc.sync.dma_start(bgr[:], bg_d[:])
            asr = pers.tile([128, HID], fp16)
            adr = pers.tile([128, HID], fp16)
            idxd = pers.tile([128, NCH * 8], i16)
            nc.sync.dma_start(idxd[:], idxd_d[:])
            wc1 = pers.tile([128, 2 * 128], fp16)
            wc2 = pers.tile([128, 64], fp32)
            wc3 = pers.tile([64, 8], fp32)
            bc1 = pers.tile([128, 1], fp32); nc.sync.dma_start(bc1[:], bc1_d[:])
            bc2 = pers.tile([64, 1], fp32); nc.sync.dma_start(bc2[:], bc2_d[:])
            bc3 = pers.tile([8, 1], fp32); nc.sync.dma_start(bc3[:], bc3_d[:])
            rowmask = pers.tile([128, 1], fp32); nc.sync.dma_start(rowmask[:], rowmask_d[:])

            iota_i = pers.tile([128, 128], i16)
            nc.gpsimd.iota(iota_i[:], pattern=[[1, 128]], base=0, channel_multiplier=0)
            iota_f = pers.tile([128, 128], fp16)
            nc.vector.tensor_copy(iota_f[:], iota_i[:])
            iop_i = pers.tile([128, 1], i16)
            nc.gpsimd.iota(iop_i[:], pattern=[[1, 1]], base=0, channel_multiplier=1)
            iop_f = pers.tile([128, 1], fp16)
            nc.vector.tensor_copy(iop_f[:], iop_i[:])
            ident = pers.tile([128, 128], fp16)
            nc.vector.tensor_tensor(
                ident[:], iop_f[:].broadcast_to([128, 128]), iota_f[:],
                op=ALU.is_equal)
            ones_r = pers.tile([1, 128], fp16)
            nc.vector.memset(ones_r[:], 1.0)
            ones_c = pers.tile([128, 1], fp16)
            nc.vector.memset(ones_c[:], 1.0)

            als_all = pers.tile([128, NB, HEADS], fp32)
            ald_all = pers.tile([128, NB, HEADS], fp32)
            crep = pers.tile([128, HEADS], fp32)

            # self-loop stashes: table row of each local node, pre-scaled by
            # dinv[d] (so evict just adds them to the PSUM aggregate).
            st1 = pers.tile([128, NB, D_IN], fp8)
            st2 = pers.tile([128, NB, HID], fp8)
            st3 = pers.tile([128, NB, H3D], fp8)
            sthg = pers.tile([128, NB, HID], fp8)

            h1_all = pers.tile([128, NB, HID], fp16)

            def rows(b):
                return LASTB if b == NB - 1 else 128

            # ---------- helpers ----------
            def transpose_to_sbuf(pool, psum_pool, src16, nslab, tag):
                """src16 [128, nslab*128] fp16 -> returns [128, nslab, 128] fp16."""
                out = pool.tile([128, nslab, 128], fp16, tag=tag, name=f"tT_{tag}")
                for s in range(nslab):
                    pt = psum_pool.tile([128, 128], fp16, tag="tr", name="pt_tr", bufs=2)
                    nc.tensor.transpose(pt[:], src16[:, s * 128:(s + 1) * 128], ident[:])
                    nc.scalar.copy(out[:, s, :], pt[:])
                return out

            qctr = [0]

            def next_q():
                qctr[0] += 1
                return qctr[0] % 4

            def gather_into(m_tile, table, start_chunk, n_chunks, elem, it=None):
                it = idxs if it is None else it
                nc.gpsimd.dma_gather(
                    m_tile[:, 0:n_chunks, :], table,
                    it[:, start_chunk * 8:(start_chunk + n_chunks) * 8],
                    num_idxs=n_chunks * 128, num_idxs_reg=n_chunks * 128,
                    elem_size=elem, single_packet=False, queue_num=next_q())

            def build_S(pool, k0, n_chunks, tag, dtype=fp16, dr=None):
                dr = dstrel if dr is None else dr
                S = pool.tile([128, n_chunks, 128], dtype, tag=tag, name=f"S_{tag}")
                for kk in range(n_chunks):
                    nc.vector.tensor_scalar(
                        S[:, kk, :], iota_f[:], dr[:, k0 + kk:k0 + kk + 1],
                        None, op0=ALU.is_equal)
                return S

            def maybe_cc(kind, op, replica_groups, ins, outs):
                if no_cc:
                    nc.sync.dma_start(outs[0].tensor[0:ins[0].shape[0]], ins[0])
                else:
                    nc.gpsimd.collective_compute(kind, op, replica_groups=replica_groups,
                                                 ins=ins, outs=outs)

            def run_body(rep):
                # ===== phase 0: g1 = dinv * x, st1 = dinv * g1, batched =====
                with tc.tile_pool(name=f"p0_{rep}", bufs=2) as p0:
                    for b0 in range(0, NB, PH0B):
                        nb_b = min(PH0B, NB - b0)
                        full = nb_b if b0 + nb_b < NB else nb_b - 1
                        xt = p0.tile([128, PH0B, D_IN], fp32, name="xt")
                        gt = p0.tile([128, PH0B, D_IN], fp16, name="gt")
                        if b0 + nb_b == NB:
                            nc.vector.memset(xt[:, nb_b - 1, :], 0.0)
                        r_tot = (nb_b - 1) * 128 + rows(b0 + nb_b - 1)
                        src_ap = xs[b0 * 128:b0 * 128 + r_tot, :]
                        if r_tot == nb_b * 128:
                            nc.sync.dma_start(
                                xt[:, 0:nb_b, :],
                                src_ap.rearrange("(b p) d -> p b d", p=128))
                        else:
                            if nb_b > 1:
                                nc.sync.dma_start(
                                    xt[:, 0:nb_b - 1, :],
                                    xs[b0 * 128:(b0 + nb_b - 1) * 128, :]
                                    .rearrange("(b p) d -> p b d", p=128))
                            nc.sync.dma_start(
                                xt[:LASTB, nb_b - 1, :],
                                xs[(NB - 1) * 128:NPC, :])
                        for j in range(nb_b):
                            b = b0 + j
                            nc.vector.tensor_scalar(
                                gt[:, j, :], xt[:, j, :], dinv[:, b:b + 1],
                                None, op0=ALU.mult)
                            nc.vector.tensor_scalar(
                                st1[:, b, :], gt[:, j, :], dinv[:, b:b + 1],
                                None, op0=ALU.mult)
                        if full > 0:
                            nc.sync.dma_start(
                                g1loc[b0 * 128:(b0 + full) * 128, :]
                                .rearrange("(b p) d -> p b d", p=128),
                                gt[:, 0:full, :])
                        if full < nb_b:
                            nc.sync.dma_start(
                                g1loc[(NB - 1) * 128:NPC, :], gt[:LASTB, nb_b - 1, :])

                if rep == 0:
                    nc.sync.dma_start(asr[:], asrc_d[:])
                    nc.sync.dma_start(adr[:], adst_d[:])
                    nc.sync.dma_start(wc1[:], wc1_d[:])
                    nc.sync.dma_start(wc2[:], wc2_d[:])
                    nc.sync.dma_start(wc3[:], wc3_d[:])
                maybe_cc("AllGather", ALU.bypass, RG, [g1loc[:]], [g1full[:]])

                # ================= GCN layer runner =================
                def gcn_layer(lname, table_full, D, evict_fn):
                    with (tc.tile_pool(name=f"{lname}_sb_{rep}", bufs=2) as lp,
                          tc.tile_pool(name=f"{lname}_ps_{rep}", bufs=5, space="PSUM") as pp,
                          tc.tile_pool(name=f"{lname}_wps_{rep}", bufs=2, space="PSUM") as wp):
                        tab_lo = table_full[0:HALF, :]
                        tab_hi = table_full[HALF:N, :]
                        for (lo_s, lo_n, hi_s, hi_n, g) in layout:
                            paggs = {}
                            for b in g:
                                paggs[b] = pp.tile([128, D], fp32, tag="agg", name="pagg")
                            first = {b: True for b in g}
                            total = {b: 0 for b in g}
                            for b in g:
                                total[b] = int(sched["CH"][b, 0] + sched["CH"][b, 1])
                            done = {b: 0 for b in g}
                            for (start, n_ch, tab) in ((lo_s, lo_n, tab_lo), (hi_s, hi_n, tab_hi)):
                                if n_ch == 0:
                                    continue
                                m = lp.tile([128, n_ch, D], fp16, tag="m", name="m", bufs=3)
                                gather_into(m, tab, start, n_ch, D)
                                S = build_S(lp, start, n_ch, "s")
                                for kk in range(n_ch):
                                    b = chunk_block[start + kk]
                                    done[b] += 1
                                    nc.tensor.matmul(
                                        paggs[b][:], S[:, kk, :], m[:, kk, :],
                                        start=first[b], stop=(done[b] == total[b]))
                                    first[b] = False
                            for b in g:
                                evict_fn(b, paggs[b], lp, wp)

                # ---------- layer 1 ----------
                def evict1(b, pagg, lp, wp):
                    r = rows(b)
                    a1s = lp.tile([128, D_IN], fp16, tag="ev1", name="a1s")
                    nc.vector.scalar_tensor_tensor(
                        a1s[:], pagg[:], dinv[:, b:b + 1], st1[:, b, :],
                        op0=ALU.mult, op1=ALU.add)
                    a1T = transpose_to_sbuf(lp, wp, a1s, 1, "ev1T")
                    ph = wp.tile([128, HID], fp32, tag="wout", name="ph1", bufs=1)
                    nc.tensor.matmul(ph[:], a1T[:, 0, :], w1[:], start=True, stop=False)
                    nc.tensor.matmul(ph[:], ones_r[:], b1r[:], start=False, stop=True)
                    h1t = h1_all[:, b, :]
                    nc.scalar.activation(h1t, ph[:], ACT.Relu)
                    g2t = lp.tile([128, HID], fp16, tag="ev1g", name="g2t")
                    nc.vector.tensor_scalar(g2t[:], h1t, dinv[:, b:b + 1], None, op0=ALU.mult)
                    nc.vector.tensor_scalar(st2[:, b, :], g2t[:], dinv[:, b:b + 1],
                                            None, op0=ALU.mult)
                    nc.sync.dma_start(g2loc[b * 128:b * 128 + r, :], g2t[:r, :])

                gcn_layer("L1", g1full, D_IN, evict1)
                maybe_cc("AllGather", ALU.bypass, RG, [g2loc[:]], [g2full[:]])

                # ---------- layer 2 (+ residual + L3 transform) ----------
                def evict2(b, pagg, lp, wp):
                    r = rows(b)
                    a2s = lp.tile([128, HID], fp16, tag="ev2", name="a2s")
                    nc.vector.scalar_tensor_tensor(
                        a2s[:], pagg[:], dinv[:, b:b + 1], st2[:, b, :],
                        op0=ALU.mult, op1=ALU.add)
                    a2T = transpose_to_sbuf(lp, wp, a2s, 2, "ev2T")
                    ph = wp.tile([128, HID], fp32, tag="wout", name="ph2", bufs=1)
                    nc.tensor.matmul(ph[:], a2T[:, 0, :], w2[:, 0:HID], start=True, stop=False)
                    nc.tensor.matmul(ph[:], a2T[:, 1, :], w2[:, HID:2 * HID], start=False, stop=False)
                    nc.tensor.matmul(ph[:], ones_r[:], b2r[:], start=False, stop=True)
                    r2 = lp.tile([128, HID], fp32, tag="ev2r", name="r2")
                    nc.scalar.activation(r2[:], ph[:], ACT.Relu)
                    h2t = lp.tile([128, HID], fp32, tag="ev2h", name="h2t")
                    nc.vector.tensor_tensor(h2t[:], r2[:], h1_all[:, b, :], op=ALU.add)
                    g3t = lp.tile([128, H3D], fp16, tag="ev2g", name="g3t")
                    nc.vector.tensor_scalar(g3t[:], h2t[:], dinv[:, b:b + 1], None, op0=ALU.mult)
                    nc.vector.tensor_scalar(st3[:, b, :], g3t[:], dinv[:, b:b + 1],
                                            None, op0=ALU.mult)
                    nc.sync.dma_start(g3loc[b * 128:b * 128 + r, :], g3t[:r, :])

                gcn_layer("L2", g2full, HID, evict2)
                maybe_cc("AllGather", ALU.bypass, RG, [g3loc[:]], [g3full[:]])

                # ---------- layer 3 aggregation + GAT prep ----------
                def evict3(b, pagg, lp, wp):
                    r = rows(b)
                    a3s = lp.tile([128, H3D], fp16, tag="ev3b", name="a3s")
                    nc.vector.scalar_tensor_tensor(
                        a3s[:], pagg[:], dinv[:, b:b + 1], st3[:, b, :],
                        op0=ALU.mult, op1=ALU.add)
                    a3T = transpose_to_sbuf(lp, wp, a3s, 2, "ev3aT")
                    p3 = wp.tile([128, 128], fp32, tag="wout", name="p3", bufs=1)
                    nc.tensor.matmul(p3[:], a3T[:, 0, :], w3[:, 0:128], start=True, stop=False)
                    nc.tensor.matmul(p3[:], a3T[:, 1, :], w3[:, 128:256], start=False, stop=True)
                    a3b = lp.tile([128, 128], fp32, tag="ev3c", name="a3b")
                    nc.vector.tensor_tensor(a3b[:], p3[:], b3r[:], op=ALU.add)
                    h3t16 = lp.tile([128, 128], fp16, tag="ev3h", name="h3t16")
                    nc.scalar.activation(h3t16[:], a3b[:], ACT.Relu)
                    h3T = transpose_to_sbuf(lp, wp, h3t16, 1, "ev3T")
                    phg = wp.tile([128, HID], fp32, tag="wout", name="phg", bufs=1)
                    nc.tensor.matmul(phg[:], h3T[:, 0, :], wg[:], start=True, stop=True)
                    # al_s / al_d
                    ts1 = lp.tile([128, HID], fp32, tag="ev3t1", name="ts1")
                    nc.vector.tensor_tensor(ts1[:], phg[:], asr[:], op=ALU.mult)
                    nc.vector.tensor_reduce(
                        als_all[:, b, :], ts1[:].rearrange("p (h f) -> p h f", h=HEADS),
                        axis=mybir.AxisListType.X, op=ALU.add)
                    ts2 = lp.tile([128, HID], fp32, tag="ev3t2", name="ts2")
                    nc.vector.tensor_tensor(ts2[:], phg[:], adr[:], op=ALU.mult)
                    nc.vector.tensor_reduce(
                        ald_all[:, b, :], ts2[:].rearrange("p (h f) -> p h f", h=HEADS),
                        axis=mybir.AxisListType.X, op=ALU.add)
                    # table tile: [hg fp8 (256) | al_s f32 (4) | pad]
                    tabt = lp.tile([128, GROW], fp8, tag="ev3tab", name="tabt")
                    nc.vector.tensor_copy(tabt[:, 0:HID], phg[:])
                    nc.vector.tensor_copy(sthg[:, b, :], tabt[:, 0:HID])
                    nc.scalar.copy(tabt[:, HID:HID + 16].bitcast(fp32), als_all[:, b, :])
                    nc.sync.dma_start(gtloc[b * 128:b * 128 + r, :], tabt[:r, :])
                    ald16 = lp.tile([128, HEADS], fp16, tag="ev3a", name="ald16")
                    nc.vector.tensor_copy(ald16[:], ald_all[:, b, :])
                    nc.sync.dma_start(aldtab[b * 128:b * 128 + r, 0:HEADS], ald16[:r, :])

                gcn_layer("L3", g3full, H3D, evict3)

                maybe_cc("AllGather", ALU.bypass, RG, [gtloc[:]], [gtfull[:]])

                # shift constants c[h] = leaky(max al_s + max al_d)
                cps = contextlib.ExitStack()
                cp = cps.enter_context(tc.tile_pool(name=f"cp_{rep}", bufs=1))
                cpp = cps.enter_context(tc.tile_pool(name=f"cpp_{rep}", bufs=1, space="PSUM"))
                m1 = cp.tile([128, HEADS], fp32)
                nc.vector.tensor_reduce(
                    m1[:], als_all[:].rearrange("p b h -> p h b"),
                    axis=mybir.AxisListType.X, op=ALU.max)
                m2 = cp.tile([128, HEADS], fp32)
                nc.vector.tensor_reduce(
                    m2[:], ald_all[:].rearrange("p b h -> p h b"),
                    axis=mybir.AxisListType.X, op=ALU.max)
                m1_16 = cp.tile([128, HEADS], fp16)
                nc.vector.tensor_copy(m1_16[:], m1[:])
                m2_16 = cp.tile([128, HEADS], fp16)
                nc.vector.tensor_copy(m2_16[:], m2[:])
                pmt1 = cpp.tile([HEADS, 128], fp16, tag="pmt1", name="pmt1")
                nc.tensor.transpose(pmt1[:], m1_16[:], ident[:])
                pmt2 = cpp.tile([HEADS, 128], fp16, tag="pmt2", name="pmt2")
                nc.tensor.transpose(pmt2[:], m2_16[:], ident[:])
                mt = cp.tile([HEADS, 2 * 128], fp32)
                nc.scalar.copy(mt[:, 0:128], pmt1[:])
                nc.scalar.copy(mt[:, 128:256], pmt2[:])
                ms = cp.tile([HEADS, 2], fp32)
                nc.vector.tensor_reduce(
                    ms[:], mt[:].rearrange("p (a j) -> p a j", a=2),
                    axis=mybir.AxisListType.X, op=ALU.max)
                ub = cp.tile([HEADS, 1], fp32)
                nc.vector.tensor_tensor(ub[:], ms[:, 0:1], ms[:, 1:2], op=ALU.add)
                ub2 = cp.tile([HEADS, 1], fp32)
                nc.vector.tensor_scalar(ub2[:], ub[:], 0.2, None, op0=ALU.mult)
                cc = cp.tile([HEADS, 1], fp32)
                nc.vector.tensor_tensor(cc[:], ub[:], ub2[:], op=ALU.max)
                cc16 = cp.tile([HEADS, 1], fp16)
                nc.vector.tensor_copy(cc16[:], cc[:])
                pcr = cpp.tile([1, HEADS], fp16)
                nc.tensor.transpose(pcr[:], cc16[:HEADS, :], ident[0:HEADS, 0:HEADS])
                pcr_sb = cp.tile([1, HEADS], fp16)
                nc.scalar.copy(pcr_sb[:], pcr[:])
                pcrep = cpp.tile([128, HEADS], fp32)
                nc.tensor.matmul(pcrep[:], ones_r[:], pcr_sb[:], start=True, stop=True)
                nc.scalar.copy(crep[:], pcrep[:])
                cps.close()

                # ================= GAT layer =================
                plp_cm = tc.tile_pool(name=f"pool_ps_{rep}", bufs=1, space="PSUM")
                plp = plp_cm.__enter__()
                ppool0 = plp.tile([128, 1], fp32, tag="pp0", name="ppool0")
                ppool1 = plp.tile([128, 1], fp32, tag="pp1", name="ppool1")
                with (tc.tile_pool(name=f"gat_sb_{rep}", bufs=2) as gp,
                      tc.tile_pool(name=f"gat_ps_{rep}", bufs=6, space="PSUM") as gpp):
                    idxs_g = gp.tile([128, NCH * 8], i16, tag="gidx", name="idxs_g", bufs=1)
                    nc.sync.dma_start(idxs_g[:], idxs_g_d[:])
                    dstrel_g = gp.tile([128, NCH], fp32, tag="gdr", name="dstrel_g", bufs=1)
                    nc.sync.dma_start(dstrel_g[:], dstrel_g_d[:])
                    tab_lo = gtfull[0:HALF, :]
                    tab_hi = gtfull[HALF:N, :]
                    first = {b: True for b in range(NB)}
                    done = {b: 0 for b in range(NB)}
                    total = {b: int(sched["CH"][b, 0] + sched["CH"][b, 1]) for b in range(NB)}
                    paggs = {}

                    def gat_span(start, n_ch, tab):
                        if n_ch == 0:
                            return
                        mald = gp.tile([128, n_ch, 128], fp16, tag="gald", name="mald", bufs=2)
                        nc.gpsimd.dma_gather(
                            mald[:, 0:n_ch, :], aldtab[:],
                            idxd[:, start * 8:(start + n_ch) * 8],
                            num_idxs=n_ch * 128, num_idxs_reg=n_ch * 128,
                            elem_size=128, single_packet=False, queue_num=next_q())
                        m = gp.tile([128, n_ch, GROW], fp8, tag="gm", name="gm", bufs=3)
                        gather_into(m, tab, start, n_ch, GROW, it=idxs_g)
                        S = build_S(gp, start, n_ch, "gs", dtype=fp8, dr=dstrel_g)
                        u = gp.tile([128, n_ch * HEADS], fp32, tag="gu", name="gu")
                        nc.vector.tensor_tensor(
                            u[:].rearrange("p (k h) -> p k h", h=HEADS),
                            m[:, :, HID:HID + 16].bitcast(fp32),
                            mald[:, :, 0:HEADS], op=ALU.add)
                        # leaky_relu on DVE in place: max(u, 0.2u), minus shift
                        nc.vector.scalar_tensor_tensor(
                            u[:], u[:], NEG, u[:], op0=ALU.mult, op1=ALU.max)
                        nc.vector.tensor_tensor(
                            u[:].rearrange("p (k h) -> p k h", h=HEADS),
                            u[:].rearrange("p (k h) -> p k h", h=HEADS),
                            crep[:].unsqueeze(1).broadcast_to([128, n_ch, HEADS]),
                            op=ALU.subtract)
                        expe = gp.tile([128, n_ch, HEADS], fp16, tag="gex", name="gex")
                        nc.scalar.activation(
                            expe[:].rearrange("p k h -> p (k h)"), u[:], ACT.Exp)
                        # write exp weights as message cols + weight hg in place
                        nc.vector.tensor_copy(m[:, :, HID:HID + HEADS], expe[:])
                        nc.vector.tensor_tensor(
                            m[:, :, 0:HID].rearrange("p k (h f) -> p k h f", h=HEADS),
                            m[:, :, 0:HID].rearrange("p k (h f) -> p k h f", h=HEADS),
                            expe[:].unsqueeze(3).broadcast_to([128, n_ch, HEADS, FH]),
                            op=ALU.mult)
                        for kk in range(n_ch):
                            b = chunk_block_g[start + kk]
                            done[b] += 1
                            nc.tensor.matmul(
                                paggs[b][:], S[:, kk, :], m[:, kk, 0:GDM],
                                start=first[b], stop=(done[b] == total[b]))
                            first[b] = False

                    def gat_evict(b):
                        r = rows(b)
                        pg = paggs.pop(b)
                        # self-loop attention weight wexp = exp(leaky(als+ald)-c)
                        wu = gp.tile([128, HEADS], fp32, tag="gwu", name="gwu")
                        nc.vector.tensor_tensor(
                            wu[:], als_all[:, b, :], ald_all[:, b, :], op=ALU.add)
                        nc.vector.scalar_tensor_tensor(
                            wu[:], wu[:], NEG, wu[:], op0=ALU.mult, op1=ALU.max)
                        nc.vector.tensor_tensor(wu[:], wu[:], crep[:], op=ALU.subtract)
                        wexp = gp.tile([128, HEADS], fp16, tag="gwe", name="gwe")
                        nc.scalar.activation(wexp[:], wu[:], ACT.Exp)
                        den = gp.tile([128, HEADS], fp32, tag="gden", name="gden")
                        nc.vector.tensor_tensor(
                            den[:], pg[:, HID:HID + HEADS], wexp[:], op=ALU.add)
                        nc.vector.tensor_scalar(den[:], den[:], 1e-30, None, op0=ALU.max)
                        rden = gp.tile([128, HEADS], fp32, tag="grden", name="grden")
                        nc.vector.reciprocal(rden[:], den[:])
                        t1 = gp.tile([128, HID], fp32, tag="gt1", name="gt1")
                        for h in range(HEADS):
                            nc.vector.scalar_tensor_tensor(
                                t1[:, h * FH:(h + 1) * FH],
                                sthg[:, b, h * FH:(h + 1) * FH],
                                wexp[:, h:h + 1],
                                pg[:, h * FH:(h + 1) * FH],
                                op0=ALU.mult, op1=ALU.add)
                        nc.vector.tensor_tensor(
                            t1[:].rearrange("p (h f) -> p h f", h=HEADS),
                            t1[:].rearrange("p (h f) -> p h f", h=HEADS),
                            rden[:].unsqueeze(2).broadcast_to([128, HEADS, FH]),
                            op=ALU.mult)
                        nc.vector.tensor_tensor(t1[:], t1[:], bgr[:], op=ALU.add)
                        hatt = gp.tile([128, HID], fp16, tag="ghat", name="ghat")
                        nc.scalar.activation(hatt[:], t1[:], ACT.Relu)
                        if r < 128:
                            nc.vector.tensor_scalar(hatt[:], hatt[:], rowmask[:], None, op0=ALU.mult)
                        nc.tensor.matmul(ppool0[:], hatt[:, 0:128], ones_c[:],
                                         start=(b == 0), stop=(b == NB - 1))
                        nc.tensor.matmul(ppool1[:], hatt[:, 128:256], ones_c[:],
                                         start=(b == 0), stop=(b == NB - 1))

                    for (lo_s, lo_n, hi_s, hi_n, g) in layout_g:
                        for b in g:
                            paggs[b] = gpp.tile([128, GDM], fp32, tag="gagg", name="gagg")
                        for col, (start, n_ch) in enumerate(((lo_s, lo_n), (hi_s, hi_n))):
                            gat_span(start, n_ch, tab_lo if col == 0 else tab_hi)
                        for b in g:
                            gat_evict(b)

                # ---------- pooling + AllReduce + MLP ----------
                with (tc.tile_pool(name=f"mlp_sb_{rep}", bufs=1) as mp,
                      tc.tile_pool(name=f"mlp_ps_{rep}", bufs=1, space="PSUM") as mpp):
                    pool_sb = mp.tile([128, 2], fp32, name="pool_sb")
                    nc.scalar.copy(pool_sb[:, 0:1], ppool0[:])
                    nc.scalar.copy(pool_sb[:, 1:2], ppool1[:])
                    nc.sync.dma_start(arin[:], pool_sb[:])
                    maybe_cc("AllReduce", ALU.add, RG, [arin[:]], [arout[:]])
                    pooled = mp.tile([128, 2], fp32, name="pooled")
                    nc.sync.dma_start(pooled[:], arout[:])
                    pooled16 = mp.tile([128, 2], fp16, name="pooled16")
                    nc.vector.tensor_scalar(pooled16[:], pooled[:], 1.0 / N, None, op0=ALU.mult)
                    pz1 = mpp.tile([128, 1], fp32, tag="pz", name="pz1")
                    nc.tensor.matmul(pz1[:], wc1[:, 0:128], pooled16[:, 0:1], start=True, stop=False)
                    nc.tensor.matmul(pz1[:], wc1[:, 128:256], pooled16[:, 1:2], start=False, stop=True)
                    z1 = mp.tile([128, 1], fp32, name="z1")
                    nc.scalar.activation(z1[:], pz1[:], ACT.Relu, bias=bc1[:])
                    pz2 = mpp.tile([64, 1], fp32, tag="pz", name="pz2")
                    nc.tensor.matmul(pz2[:], wc2[:], z1[:], start=True, stop=True)
                    z2 = mp.tile([64, 1], fp32, name="z2")
                    nc.scalar.activation(z2[:], pz2[:], ACT.Relu, bias=bc2[:])
                    pz3 = mpp.tile([8, 1], fp32, tag="pz", name="pz3")
                    nc.tensor.matmul(pz3[:], wc3[:], z2[:64, :], start=True, stop=True)
                    zo = mp.tile([8, 1], fp32, name="zo")
                    nc.scalar.activation(zo[:], pz3[:], ACT.Identity, bias=bc3[:])
                    nc.sync.dma_start(out_d[:], zo[:])
                plp_cm.__exit__(None, None, None)

            for _rep in range(repeat):
                run_body(_rep)

    nc.compile()
    return nc


# --------------------------------------------------------------------------
# entry point
# --------------------------------------------------------------------------

def kernel(**inputs):
    x = np.asarray(inputs["x"], dtype=np.float32)
    ei = np.asarray(inputs["edge_index"], dtype=np.int64)
    sched = _preprocess(x, ei)
    nc = _build(sched)

    W = {k: np.asarray(v, dtype=np.float32) for k, v in inputs.items()
         if k not in ("x", "edge_index")}

    def pack_k(w, nslab):   # [K, M] -> [128, nslab*M] (row-slab packed)
        K, M = w.shape
        out = np.zeros((128, nslab * M), np.float32)
        for s in range(nslab):
            r0 = s * 128
            r1 = min(K, r0 + 128)
            out[0:r1 - r0, s * M:(s + 1) * M] = w[r0:r1]
        return out

    asrc_flat = np.tile(W["a_src"].reshape(1, HEADS * FH), (128, 1))
    adst_flat = np.tile(W["a_dst"].reshape(1, HEADS * FH), (128, 1))

    common = {
        "w1_d": pack_k(W["W1"], 1).astype(F16),
        "w2_d": pack_k(W["W2"], 2).astype(F16),
        "w3_d": pack_k(W["W3"], 2).astype(F16),
        "wg_d": pack_k(W["Wg"], 1).astype(F16),
        "b1_d": W["b1"].reshape(1, -1).astype(F16),
        "b2_d": W["b2"].reshape(1, -1).astype(F16),
        "b3_d": np.tile(W["b3"].reshape(1, -1), (128, 1)).astype(F16),
        "bg_d": np.tile(W["bg"].reshape(1, -1), (128, 1)).astype(F16),
        "asrc_d": asrc_flat.astype(F16),
        "adst_d": adst_flat.astype(F16),
        "wc1_d": pack_k(W["Wc1"], 2).astype(F16),
        "wc2_d": pack_k(W["Wc2"], 1)[:, :64].astype(np.float32),
        "wc3_d": pack_k(W["Wc3"], 1)[:64, :8].astype(np.float32),
        "bc1_d": W["bc1"].reshape(-1, 1).astype(np.float32),
        "bc2_d": W["bc2"].reshape(-1, 1).astype(np.float32),
        "bc3_d": W["bc3"].reshape(-1, 1).astype(np.float32),
        "rowmask_d": (np.arange(128) < LASTB).astype(np.float32).reshape(128, 1),
    }

    NCH = sched["NCH"]
    in_maps = []
    for c in range(NCORES):
        rel = sched["dstrel"][c]
        in_maps.append(dict(
            common,
            xs=np.ascontiguousarray(x[c * NPC:(c + 1) * NPC]),
            idxs_d=sched["idxs"][c],
            idxd_d=sched["idxd"][c],
            idxs_g_d=sched["idxs_g"][c],
            dstrel_g_d=np.ascontiguousarray(
                sched["dstrel_g"][c].reshape(NCH, 128).T).astype(np.float32),
            dstrel_d=np.ascontiguousarray(rel.reshape(NCH, 128).T).astype(np.float32),
            dinv_d=sched["dinv"][c],
        ))

    res = run_bass_kernel_spmd(nc, in_maps, core_ids=list(range(NCORES)))
    global LAST_RESULT
    LAST_RESULT = res
    return res.results[0]["out_d"].reshape(1, OUT).astype(np.float32)


LAST_RESULT = None
